# revision 21
# baseline (speedup 1.0000x reference)
"""Causal attention (out, p_attn) Bass/Tile kernel for 8 TRN2 NeuronCores. v6"""
import sys

sys.path.insert(0, "/opt/trn_rl_repo")

from contextlib import ExitStack

import numpy as np

from concourse import bacc, mybir
from concourse.bass_utils import run_bass_kernel_spmd
from concourse.masks import make_identity
from concourse.tile import TileContext
from concourse.tile_rust import add_dep_helper

B, H, S, DK = 2, 16, 2048, 64
NCORES = 8
HPC = (B * H) // NCORES
NQT = S // 128
SCALE = 1.0 / np.sqrt(DK)
F32 = mybir.dt.float32
F32R = mybir.dt.float32r
CHUNK = 512
NCH = S // CHUNK
NFILL = 0  # filler matmuls per pair-iter


def _build():
    nc = bacc.Bacc("TRN2", target_bir_lowering=False, debug=False, num_devices=NCORES)

    q_in = nc.dram_tensor("q", [HPC, S, DK], F32, kind="ExternalInput").ap()
    k_in = nc.dram_tensor("k", [HPC, S, DK], F32, kind="ExternalInput").ap()
    v_in = nc.dram_tensor("v", [HPC, S, DK], F32, kind="ExternalInput").ap()
    tm_in = nc.dram_tensor("trimask", [128, 128], F32, kind="ExternalInput").ap()
    p_out = nc.dram_tensor("p_attn", [HPC, S, S], F32, kind="ExternalOutput").ap()
    o_out = nc.dram_tensor("o", [HPC, S, DK], F32, kind="ExternalOutput").ap()

    with TileContext(nc) as tc, ExitStack() as ctx:
        sb = ctx.enter_context(tc.tile_pool(name="sb", bufs=1))
        raw = ctx.enter_context(tc.tile_pool(name="raw", bufs=2))
        dmaj = ctx.enter_context(tc.tile_pool(name="dmaj", bufs=4))
        epool = ctx.enter_context(tc.tile_pool(name="epool", bufs=6))
        etpool = ctx.enter_context(tc.tile_pool(name="etpool", bufs=20))
        small = ctx.enter_context(tc.tile_pool(name="small", bufs=8))
        outp = ctx.enter_context(tc.tile_pool(name="outp", bufs=2))
        mm_ps = ctx.enter_context(tc.tile_pool(name="mm_ps", bufs=5, space="PSUM"))
        pv_ps = ctx.enter_context(tc.tile_pool(name="pv_ps", bufs=1, space="PSUM"))
        tp_ps = ctx.enter_context(tc.tile_pool(name="tp_ps", bufs=2, space="PSUM"))

        ident = sb.tile([128, 128], F32, tag="ident")
        make_identity(nc, ident[:])
        trimask = sb.tile([128, 128], F32, tag="trimask")
        nc.sync.dma_start(out=trimask[:], in_=tm_in[:])

        last_pe = [None]

        def order(bi, after=None):
            # only enforce pair adjacency: B scheduled right after A so the
            # row-packed halves run concurrently on disjoint PE row-groups
            if after is not None:
                add_dep_helper(bi.ins, after.ins, sync=False, reason="pair-adj")
            return bi

        def filler(n=NFILL):
            pass

        # ---- prologue: all heads' loads + transposes ----
        qt2s, kt2s, vrs = [], [], []
        for h in range(HPC):
            qk_raw = raw.tile([128, NQT, 2 * DK], F32, tag="qk_raw")
            v_raw = raw.tile([128, NQT, DK], F32, tag="v_raw")
            nc.sync.dma_start(out=qk_raw[:, :, 0:DK],
                              in_=q_in[h].rearrange("(n p) d -> p n d", p=128))
            nc.sync.dma_start(out=qk_raw[:, :, DK:2 * DK],
                              in_=k_in[h].rearrange("(n p) d -> p n d", p=128))
            nc.sync.dma_start(out=v_raw[:], in_=v_in[h].rearrange("(n p) d -> p n d", p=128))

            qt2 = dmaj.tile([128, S], F32R, tag="qt2")
            kt2 = dmaj.tile([128, S], F32R, tag="kt2")
            vr = dmaj.tile([128, NQT, DK], F32R, tag="vr")
            nc.vector.tensor_copy(vr[:], v_raw[:])
            for t in range(NQT):
                tqk = tp_ps.tile([128, 128], F32, tag="tp")
                order(nc.tensor.transpose(tqk[:], qk_raw[:, t, :], ident[:]))
                nc.scalar.copy(qt2[0:64, t * 128:(t + 1) * 128], tqk[0:64, :])
                nc.vector.tensor_copy(kt2[64:128, t * 128:(t + 1) * 128], tqk[64:128, :])
            nc.sync.dma_start(out=qt2[64:128, :], in_=qt2[0:64, :])
            nc.sync.dma_start(out=kt2[0:64, :], in_=kt2[64:128, :])
            qt2s.append(qt2)
            kt2s.append(kt2)
            vrs.append(vr)

        for h in range(HPC):
            qt2, kt2, vr = qt2s[h], kt2s[h], vrs[h]
            rc = small.tile([128, NQT], F32, tag="rc")
            out_sb = outp.tile([128, NQT, DK], F32, tag="out_sb")

            for qc in range(NCH):
                # ========== E path: q-tiles 4qc .. 4qc+3 ==========
                e_tiles = {}
                for pair in (2 * qc, 2 * qc + 1):
                    qtA, qtB = 2 * pair, 2 * pair + 1
                    extB = (qtB + 1) * 128
                    eA = epool.tile([128, S], F32, tag="e")
                    eB = epool.tile([128, S], F32, tag="e")
                    e_tiles[qtA], e_tiles[qtB] = eA, eB
                    partsA = small.tile([128, 8], F32, tag="parts")
                    partsB = small.tile([128, 8], F32, tag="parts")
                    npA = npB = 0
                    for c in range(0, extB, CHUNK):
                        w = min(CHUNK, extB - c)
                        psA = mm_ps.tile([128, CHUNK], F32, tag="mm")
                        psB = mm_ps.tile([128, CHUNK], F32, tag="mm")
                        mA = order(nc.tensor.matmul(
                            psA[:, 0:w], qt2[0:64, qtA * 128:(qtA + 1) * 128],
                            kt2[0:64, c:c + w], start=True, stop=True))
                        order(nc.tensor.matmul(
                            psB[:, 0:w], qt2[64:128, qtB * 128:(qtB + 1) * 128],
                            kt2[64:128, c:c + w], start=True, stop=True), after=mA)
                        for qt, ps, e, parts in ((qtA, psA, eA, partsA),
                                                 (qtB, psB, eB, partsB)):
                            ext = (qt + 1) * 128
                            dlo = ext - 128
                            if c <= dlo < c + w:
                                nc.vector.tensor_add(ps[:, dlo - c:dlo - c + 128],
                                                     ps[:, dlo - c:dlo - c + 128],
                                                     trimask[:])
                            lo, hi = c, min(c + w, ext)
                            if hi > lo:
                                i = npA if qt == qtA else npB
                                nc.scalar.activation(e[:, lo:hi], ps[:, lo - c:hi - c],
                                                     mybir.ActivationFunctionType.Exp,
                                                     scale=SCALE,
                                                     accum_out=parts[:, i:i + 1])
                                if qt == qtA:
                                    npA += 1
                                else:
                                    npB += 1
                        filler()
                    for qt, parts, np_ in ((qtA, partsA, npA), (qtB, partsB, npB)):
                        rsum = small.tile([128, 1], F32, tag="rsum")
                        nc.vector.reduce_sum(rsum[:], parts[:, 0:np_],
                                             axis=mybir.AxisListType.X)
                        nc.vector.reciprocal(rc[:, qt:qt + 1], rsum[:])

                # ========== E^T path ==========
                nkt = 4 * qc + 4
                ets, offs = {}, {}
                for j in range(nkt // 2):
                    ktA, ktB = 2 * j, 2 * j + 1
                    c0 = max(0, ktA * 128 - qc * CHUNK)
                    w = CHUNK - c0
                    psA = mm_ps.tile([128, CHUNK], F32, tag="mm")
                    psB = mm_ps.tile([128, CHUNK], F32, tag="mm")
                    mA = order(nc.tensor.matmul(
                        psA[:, 0:w], kt2[0:64, ktA * 128:(ktA + 1) * 128],
                        qt2[0:64, qc * CHUNK + c0:(qc + 1) * CHUNK],
                        start=True, stop=True))
                    order(nc.tensor.matmul(
                        psB[:, 0:w], kt2[64:128, ktB * 128:(ktB + 1) * 128],
                        qt2[64:128, qc * CHUNK + c0:(qc + 1) * CHUNK],
                        start=True, stop=True), after=mA)
                    for kt, ps in ((ktA, psA), (ktB, psB)):
                        et = etpool.tile([128, CHUNK], F32R, tag="et")
                        ets[kt], offs[kt] = et, c0
                        nc.scalar.activation(et[:, c0:CHUNK], ps[:, 0:w],
                                             mybir.ActivationFunctionType.Exp,
                                             scale=SCALE)
                        if kt * 128 >= qc * CHUNK:
                            off = kt * 128 - qc * CHUNK
                            nc.gpsimd.affine_select(
                                out=et[:, c0:CHUNK], in_=et[:, c0:CHUNK],
                                compare_op=mybir.AluOpType.is_ge, fill=0.0,
                                base=c0 - off, pattern=[[1, w]], channel_multiplier=-1)
                    filler()
                # PV chain (contiguous, warms/keeps HAM)
                po = pv_ps.tile([64, CHUNK], F32, tag="pv")
                for kt in range(nkt):
                    c0 = max(offs[kt], kt * 128 - qc * CHUNK)
                    order(nc.tensor.matmul(po[0:64, c0:CHUNK], vr[:, kt, :],
                                           ets[kt][:, c0:CHUNK],
                                           start=(kt == 0), stop=(kt == nkt - 1)))

                ot = outp.tile([64, CHUNK], F32, tag="ot")
                nc.vector.tensor_copy(ot[:], po[:])
                for jj in range(4):
                    qt = 4 * qc + jj
                    tp2 = tp_ps.tile([128, 64], F32, tag="tp")
                    order(nc.tensor.transpose(tp2[:], ot[:, jj * 128:(jj + 1) * 128],
                                              ident[0:64, 0:64]))
                    nc.vector.tensor_scalar_mul(out_sb[:, qt, :], tp2[:], rc[:, qt:qt + 1])

                for jj in range(4):
                    qt = 4 * qc + jj
                    ext = (qt + 1) * 128
                    e = e_tiles[qt]
                    nc.vector.tensor_scalar_mul(e[:, 0:ext], e[:, 0:ext],
                                                rc[:, qt:qt + 1])
                    nc.sync.dma_start(out=p_out[h, qt * 128:(qt + 1) * 128, 0:ext],
                                      in_=e[:, 0:ext])

            nc.sync.dma_start(out=o_out[h].rearrange("(n p) d -> p n d", p=128),
                              in_=out_sb[:])

    nc.compile()
    return nc


_NC_CACHE = None


def _get_nc():
    global _NC_CACHE
    if _NC_CACHE is None:
        _NC_CACHE = _build()
    return _NC_CACHE


def _run(query, key, value, trace=False):
    nc = _get_nc()
    q = np.ascontiguousarray(np.asarray(query, dtype=np.float32).reshape(B * H, S, DK))
    k = np.ascontiguousarray(np.asarray(key, dtype=np.float32).reshape(B * H, S, DK))
    v = np.ascontiguousarray(np.asarray(value, dtype=np.float32).reshape(B * H, S, DK))
    tm = np.where(np.arange(128)[None, :] <= np.arange(128)[:, None],
                  np.float32(0.0), np.float32(-1.0e9)).astype(np.float32)
    in_maps = [
        {"q": q[c * HPC:(c + 1) * HPC], "k": k[c * HPC:(c + 1) * HPC],
         "v": v[c * HPC:(c + 1) * HPC], "trimask": tm}
        for c in range(NCORES)
    ]
    res = run_bass_kernel_spmd(nc, in_maps, list(range(NCORES)), trace=trace)
    outs = np.stack([res.results[c]["o"] for c in range(NCORES)])
    ps = np.stack([res.results[c]["p_attn"] for c in range(NCORES)])
    return (outs.reshape(B, H, S, DK), ps.reshape(B, H, S, S)), res


def kernel(query, key, value, mask=None, **_ignored):
    (out, p_attn), _ = _run(query, key, value, trace=False)
    return out, p_attn


# revision 23
# speedup vs baseline: 1.0057x; 1.0057x over previous
"""Causal attention (out, p_attn) Bass/Tile kernel for 8 TRN2 NeuronCores. v6"""
import sys

sys.path.insert(0, "/opt/trn_rl_repo")

from contextlib import ExitStack

import numpy as np

from concourse import bacc, mybir
from concourse.bass_utils import run_bass_kernel_spmd
from concourse.masks import make_identity
from concourse.tile import TileContext
from concourse.tile_rust import add_dep_helper

B, H, S, DK = 2, 16, 2048, 64
NCORES = 8
HPC = (B * H) // NCORES
NQT = S // 128
SCALE = 1.0 / np.sqrt(DK)
F32 = mybir.dt.float32
F32R = mybir.dt.float32r
CHUNK = 512
NCH = S // CHUNK
NFILL = 0  # filler matmuls per pair-iter


def _build():
    nc = bacc.Bacc("TRN2", target_bir_lowering=False, debug=False, num_devices=NCORES)

    q_in = nc.dram_tensor("q", [HPC, S, DK], F32, kind="ExternalInput").ap()
    k_in = nc.dram_tensor("k", [HPC, S, DK], F32, kind="ExternalInput").ap()
    v_in = nc.dram_tensor("v", [HPC, S, DK], F32, kind="ExternalInput").ap()
    tm_in = nc.dram_tensor("trimask", [128, 128], F32, kind="ExternalInput").ap()
    p_out = nc.dram_tensor("p_attn", [HPC, S, S], F32, kind="ExternalOutput").ap()
    o_out = nc.dram_tensor("o", [HPC, S, DK], F32, kind="ExternalOutput").ap()

    with TileContext(nc) as tc, ExitStack() as ctx:
        sb = ctx.enter_context(tc.tile_pool(name="sb", bufs=1))
        raw = ctx.enter_context(tc.tile_pool(name="raw", bufs=2))
        dmaj = ctx.enter_context(tc.tile_pool(name="dmaj", bufs=4))
        epool = ctx.enter_context(tc.tile_pool(name="epool", bufs=6))
        etpool = ctx.enter_context(tc.tile_pool(name="etpool", bufs=16))
        small = ctx.enter_context(tc.tile_pool(name="small", bufs=8))
        outp = ctx.enter_context(tc.tile_pool(name="outp", bufs=2))
        mm_ps = ctx.enter_context(tc.tile_pool(name="mm_ps", bufs=5, space="PSUM"))
        pv_ps = ctx.enter_context(tc.tile_pool(name="pv_ps", bufs=1, space="PSUM"))
        tp_ps = ctx.enter_context(tc.tile_pool(name="tp_ps", bufs=2, space="PSUM"))

        ident = sb.tile([128, 128], F32, tag="ident")
        make_identity(nc, ident[:])
        trimask = sb.tile([128, 128], F32, tag="trimask")
        nc.sync.dma_start(out=trimask[:], in_=tm_in[:])

        last_pe = [None]

        def order(bi, after=None):
            # only enforce pair adjacency: B scheduled right after A so the
            # row-packed halves run concurrently on disjoint PE row-groups
            if after is not None:
                add_dep_helper(bi.ins, after.ins, sync=False, reason="pair-adj")
            return bi

        def filler(n=NFILL):
            pass

        # ---- prologue: all heads' loads + transposes ----
        qt2s, kt2s, vrs = [], [], []
        for h in range(HPC):
            qk_raw = raw.tile([128, NQT, 2 * DK], F32, tag="qk_raw")
            v_raw = raw.tile([128, NQT, DK], F32, tag="v_raw")
            nc.sync.dma_start(out=qk_raw[:, :, 0:DK],
                              in_=q_in[h].rearrange("(n p) d -> p n d", p=128))
            nc.sync.dma_start(out=qk_raw[:, :, DK:2 * DK],
                              in_=k_in[h].rearrange("(n p) d -> p n d", p=128))
            nc.sync.dma_start(out=v_raw[:], in_=v_in[h].rearrange("(n p) d -> p n d", p=128))

            qt2 = dmaj.tile([128, S], F32R, tag="qt2")
            kt2 = dmaj.tile([128, S], F32R, tag="kt2")
            vr = dmaj.tile([128, NQT, DK], F32R, tag="vr")
            nc.vector.tensor_copy(vr[:], v_raw[:])
            for t in range(NQT):
                tqk = tp_ps.tile([128, 128], F32, tag="tp")
                order(nc.tensor.transpose(tqk[:], qk_raw[:, t, :], ident[:]))
                nc.scalar.copy(qt2[0:64, t * 128:(t + 1) * 128], tqk[0:64, :])
                nc.vector.tensor_copy(kt2[64:128, t * 128:(t + 1) * 128], tqk[64:128, :])
            nc.sync.dma_start(out=qt2[64:128, :], in_=qt2[0:64, :])
            nc.sync.dma_start(out=kt2[0:64, :], in_=kt2[64:128, :])
            qt2s.append(qt2)
            kt2s.append(kt2)
            vrs.append(vr)

        for h in range(HPC):
            qt2, kt2, vr = qt2s[h], kt2s[h], vrs[h]
            rc = small.tile([128, NQT], F32, tag="rc")
            out_sb = outp.tile([128, NQT, DK], F32, tag="out_sb")

            for qc in range(NCH):
                # ========== E path: q-tiles 4qc .. 4qc+3 ==========
                e_tiles = {}
                for pair in (2 * qc, 2 * qc + 1):
                    qtA, qtB = 2 * pair, 2 * pair + 1
                    extB = (qtB + 1) * 128
                    eA = epool.tile([128, S], F32, tag="e")
                    eB = epool.tile([128, S], F32, tag="e")
                    e_tiles[qtA], e_tiles[qtB] = eA, eB
                    partsA = small.tile([128, 8], F32, tag="parts")
                    partsB = small.tile([128, 8], F32, tag="parts")
                    npA = npB = 0
                    extA = extB - 128
                    for c in range(0, extB, CHUNK):
                        w = min(CHUNK, extB - c)
                        wA = max(0, min(CHUNK, extA - c))  # A's own causal extent
                        psB = mm_ps.tile([128, CHUNK], F32, tag="mm")
                        mA = None
                        if wA > 0:
                            psA = mm_ps.tile([128, CHUNK], F32, tag="mm")
                            mA = order(nc.tensor.matmul(
                                psA[:, 0:wA], qt2[0:64, qtA * 128:(qtA + 1) * 128],
                                kt2[0:64, c:c + wA], start=True, stop=True))
                        order(nc.tensor.matmul(
                            psB[:, 0:w], qt2[64:128, qtB * 128:(qtB + 1) * 128],
                            kt2[64:128, c:c + w], start=True, stop=True), after=mA)
                        branches = [(qtB, psB, eB, partsB)]
                        if wA > 0:
                            branches.insert(0, (qtA, psA, eA, partsA))
                        for qt, ps, e, parts in branches:
                            ext = (qt + 1) * 128
                            dlo = ext - 128
                            if c <= dlo < c + w:
                                nc.vector.tensor_add(ps[:, dlo - c:dlo - c + 128],
                                                     ps[:, dlo - c:dlo - c + 128],
                                                     trimask[:])
                            lo, hi = c, min(c + w, ext)
                            if hi > lo:
                                i = npA if qt == qtA else npB
                                nc.scalar.activation(e[:, lo:hi], ps[:, lo - c:hi - c],
                                                     mybir.ActivationFunctionType.Exp,
                                                     scale=SCALE,
                                                     accum_out=parts[:, i:i + 1])
                                if qt == qtA:
                                    npA += 1
                                else:
                                    npB += 1
                        filler()
                    for qt, parts, np_ in ((qtA, partsA, npA), (qtB, partsB, npB)):
                        rsum = small.tile([128, 1], F32, tag="rsum")
                        nc.vector.reduce_sum(rsum[:], parts[:, 0:np_],
                                             axis=mybir.AxisListType.X)
                        nc.vector.reciprocal(rc[:, qt:qt + 1], rsum[:])

                # ========== E^T path ==========
                nkt = 4 * qc + 4
                ets, offs = {}, {}
                for j in range(nkt // 2):
                    ktA, ktB = 2 * j, 2 * j + 1
                    c0 = max(0, ktA * 128 - qc * CHUNK)
                    w = CHUNK - c0
                    psA = mm_ps.tile([128, CHUNK], F32, tag="mm")
                    psB = mm_ps.tile([128, CHUNK], F32, tag="mm")
                    mA = order(nc.tensor.matmul(
                        psA[:, 0:w], kt2[0:64, ktA * 128:(ktA + 1) * 128],
                        qt2[0:64, qc * CHUNK + c0:(qc + 1) * CHUNK],
                        start=True, stop=True))
                    order(nc.tensor.matmul(
                        psB[:, 0:w], kt2[64:128, ktB * 128:(ktB + 1) * 128],
                        qt2[64:128, qc * CHUNK + c0:(qc + 1) * CHUNK],
                        start=True, stop=True), after=mA)
                    for kt, ps in ((ktA, psA), (ktB, psB)):
                        et = etpool.tile([128, CHUNK], F32R, tag="et")
                        ets[kt], offs[kt] = et, c0
                        nc.scalar.activation(et[:, c0:CHUNK], ps[:, 0:w],
                                             mybir.ActivationFunctionType.Exp,
                                             scale=SCALE)
                        if kt * 128 >= qc * CHUNK:
                            off = kt * 128 - qc * CHUNK
                            nc.gpsimd.affine_select(
                                out=et[:, c0:CHUNK], in_=et[:, c0:CHUNK],
                                compare_op=mybir.AluOpType.is_ge, fill=0.0,
                                base=c0 - off, pattern=[[1, w]], channel_multiplier=-1)
                    filler()
                # PV chain (contiguous, warms/keeps HAM)
                po = pv_ps.tile([64, CHUNK], F32, tag="pv")
                for kt in range(nkt):
                    c0 = max(offs[kt], kt * 128 - qc * CHUNK)
                    order(nc.tensor.matmul(po[0:64, c0:CHUNK], vr[:, kt, :],
                                           ets[kt][:, c0:CHUNK],
                                           start=(kt == 0), stop=(kt == nkt - 1)))

                ot = outp.tile([64, CHUNK], F32, tag="ot")
                nc.vector.tensor_copy(ot[:], po[:])
                for jj in range(4):
                    qt = 4 * qc + jj
                    tp2 = tp_ps.tile([128, 64], F32, tag="tp")
                    order(nc.tensor.transpose(tp2[:], ot[:, jj * 128:(jj + 1) * 128],
                                              ident[0:64, 0:64]))
                    nc.vector.tensor_scalar_mul(out_sb[:, qt, :], tp2[:], rc[:, qt:qt + 1])

                for jj in range(4):
                    qt = 4 * qc + jj
                    ext = (qt + 1) * 128
                    e = e_tiles[qt]
                    nc.vector.tensor_scalar_mul(e[:, 0:ext], e[:, 0:ext],
                                                rc[:, qt:qt + 1])
                    nc.sync.dma_start(out=p_out[h, qt * 128:(qt + 1) * 128, 0:ext],
                                      in_=e[:, 0:ext])

            nc.sync.dma_start(out=o_out[h].rearrange("(n p) d -> p n d", p=128),
                              in_=out_sb[:])

    nc.compile()
    return nc


_NC_CACHE = None


def _get_nc():
    global _NC_CACHE
    if _NC_CACHE is None:
        _NC_CACHE = _build()
    return _NC_CACHE


def _run(query, key, value, trace=False):
    nc = _get_nc()
    q = np.ascontiguousarray(np.asarray(query, dtype=np.float32).reshape(B * H, S, DK))
    k = np.ascontiguousarray(np.asarray(key, dtype=np.float32).reshape(B * H, S, DK))
    v = np.ascontiguousarray(np.asarray(value, dtype=np.float32).reshape(B * H, S, DK))
    tm = np.where(np.arange(128)[None, :] <= np.arange(128)[:, None],
                  np.float32(0.0), np.float32(-1.0e9)).astype(np.float32)
    in_maps = [
        {"q": q[c * HPC:(c + 1) * HPC], "k": k[c * HPC:(c + 1) * HPC],
         "v": v[c * HPC:(c + 1) * HPC], "trimask": tm}
        for c in range(NCORES)
    ]
    res = run_bass_kernel_spmd(nc, in_maps, list(range(NCORES)), trace=trace)
    outs = np.stack([res.results[c]["o"] for c in range(NCORES)])
    ps = np.stack([res.results[c]["p_attn"] for c in range(NCORES)])
    return (outs.reshape(B, H, S, DK), ps.reshape(B, H, S, S)), res


def kernel(query, key, value, mask=None, **_ignored):
    (out, p_attn), _ = _run(query, key, value, trace=False)
    return out, p_attn


# revision 24
# speedup vs baseline: 1.0209x; 1.0150x over previous
"""Causal attention (out, p_attn) Bass/Tile kernel for 8 TRN2 NeuronCores. v6"""
import sys

sys.path.insert(0, "/opt/trn_rl_repo")

from contextlib import ExitStack

import numpy as np

from concourse import bacc, mybir
from concourse.bass_utils import run_bass_kernel_spmd
from concourse.masks import make_identity
from concourse.tile import TileContext
from concourse.tile_rust import add_dep_helper

B, H, S, DK = 2, 16, 2048, 64
NCORES = 8
HPC = (B * H) // NCORES
NQT = S // 128
SCALE = 1.0 / np.sqrt(DK)
F32 = mybir.dt.float32
F32R = mybir.dt.float32r
CHUNK = 512
NCH = S // CHUNK
NFILL = 0  # filler matmuls per pair-iter


def _build():
    nc = bacc.Bacc("TRN2", target_bir_lowering=False, debug=False, num_devices=NCORES)

    q_in = nc.dram_tensor("q", [HPC, S, DK], F32, kind="ExternalInput").ap()
    k_in = nc.dram_tensor("k", [HPC, S, DK], F32, kind="ExternalInput").ap()
    v_in = nc.dram_tensor("v", [HPC, S, DK], F32, kind="ExternalInput").ap()
    tm_in = nc.dram_tensor("trimask", [128, 128], F32, kind="ExternalInput").ap()
    p_out = nc.dram_tensor("p_attn", [HPC, S, S], F32, kind="ExternalOutput").ap()
    o_out = nc.dram_tensor("o", [HPC, S, DK], F32, kind="ExternalOutput").ap()

    with TileContext(nc) as tc, ExitStack() as ctx:
        sb = ctx.enter_context(tc.tile_pool(name="sb", bufs=1))
        raw = ctx.enter_context(tc.tile_pool(name="raw", bufs=2))
        dmaj = ctx.enter_context(tc.tile_pool(name="dmaj", bufs=4))
        epool = ctx.enter_context(tc.tile_pool(name="epool", bufs=6))
        etpool = ctx.enter_context(tc.tile_pool(name="etpool", bufs=16))
        small = ctx.enter_context(tc.tile_pool(name="small", bufs=8))
        outp = ctx.enter_context(tc.tile_pool(name="outp", bufs=2))
        mm_ps = ctx.enter_context(tc.tile_pool(name="mm_ps", bufs=5, space="PSUM"))
        pv_ps = ctx.enter_context(tc.tile_pool(name="pv_ps", bufs=1, space="PSUM"))
        tp_ps = ctx.enter_context(tc.tile_pool(name="tp_ps", bufs=2, space="PSUM"))

        ident = sb.tile([128, 128], F32, tag="ident")
        make_identity(nc, ident[:])
        trimask = sb.tile([128, 128], F32, tag="trimask")
        nc.sync.dma_start(out=trimask[:], in_=tm_in[:])

        last_pe = [None]

        def order(bi, after=None):
            # only enforce pair adjacency: B scheduled right after A so the
            # row-packed halves run concurrently on disjoint PE row-groups
            if after is not None:
                add_dep_helper(bi.ins, after.ins, sync=False, reason="pair-adj")
            return bi

        def filler(n=NFILL):
            pass

        # ---- prologue: all heads' loads + transposes ----
        qt2s, kt2s, vrs = [], [], []
        for h in range(HPC):
            qk_raw = raw.tile([128, NQT, 2 * DK], F32, tag="qk_raw")
            v_raw = raw.tile([128, NQT, DK], F32, tag="v_raw")
            nc.sync.dma_start(out=qk_raw[:, :, 0:DK],
                              in_=q_in[h].rearrange("(n p) d -> p n d", p=128))
            nc.sync.dma_start(out=qk_raw[:, :, DK:2 * DK],
                              in_=k_in[h].rearrange("(n p) d -> p n d", p=128))
            nc.sync.dma_start(out=v_raw[:], in_=v_in[h].rearrange("(n p) d -> p n d", p=128))

            qt2 = dmaj.tile([128, S], F32R, tag="qt2")
            kt2 = dmaj.tile([128, S], F32R, tag="kt2")
            vr = dmaj.tile([128, NQT, DK], F32R, tag="vr")
            nc.vector.tensor_copy(vr[:], v_raw[:])
            for t in range(NQT):
                tqk = tp_ps.tile([128, 128], F32, tag="tp")
                order(nc.tensor.transpose(tqk[:], qk_raw[:, t, :], ident[:]))
                nc.scalar.copy(qt2[0:64, t * 128:(t + 1) * 128], tqk[0:64, :])
                nc.vector.tensor_copy(kt2[64:128, t * 128:(t + 1) * 128], tqk[64:128, :])
            nc.sync.dma_start(out=qt2[64:128, :], in_=qt2[0:64, :])
            nc.sync.dma_start(out=kt2[0:64, :], in_=kt2[64:128, :])
            qt2s.append(qt2)
            kt2s.append(kt2)
            vrs.append(vr)

        for h in range(HPC):
            qt2, kt2, vr = qt2s[h], kt2s[h], vrs[h]
            rc = small.tile([128, NQT], F32, tag="rc")
            out_sb = outp.tile([128, NQT, DK], F32, tag="out_sb")

            for qc in range(NCH):
                # ========== E path: q-tiles 4qc .. 4qc+3 ==========
                e_tiles = {}
                for pair in (2 * qc, 2 * qc + 1):
                    qtA, qtB = 2 * pair, 2 * pair + 1
                    extB = (qtB + 1) * 128
                    eA = epool.tile([128, S], F32, tag="e")
                    eB = epool.tile([128, S], F32, tag="e")
                    e_tiles[qtA], e_tiles[qtB] = eA, eB
                    partsA = small.tile([128, 8], F32, tag="parts")
                    partsB = small.tile([128, 8], F32, tag="parts")
                    npA = npB = 0
                    extA = extB - 128
                    for c in range(0, extB, CHUNK):
                        w = min(CHUNK, extB - c)
                        wA = max(0, min(CHUNK, extA - c))  # A's own causal extent
                        psB = mm_ps.tile([128, CHUNK], F32, tag="mm")
                        mA = None
                        if wA > 0:
                            psA = mm_ps.tile([128, CHUNK], F32, tag="mm")
                            mA = order(nc.tensor.matmul(
                                psA[:, 0:wA], qt2[0:64, qtA * 128:(qtA + 1) * 128],
                                kt2[0:64, c:c + wA], start=True, stop=True))
                        order(nc.tensor.matmul(
                            psB[:, 0:w], qt2[64:128, qtB * 128:(qtB + 1) * 128],
                            kt2[64:128, c:c + w], start=True, stop=True), after=mA)
                        branches = [(qtB, psB, eB, partsB)]
                        if wA > 0:
                            branches.insert(0, (qtA, psA, eA, partsA))
                        for qt, ps, e, parts in branches:
                            ext = (qt + 1) * 128
                            dlo = ext - 128
                            if c <= dlo < c + w:
                                nc.vector.tensor_add(ps[:, dlo - c:dlo - c + 128],
                                                     ps[:, dlo - c:dlo - c + 128],
                                                     trimask[:])
                            lo, hi = c, min(c + w, ext)
                            if hi > lo:
                                i = npA if qt == qtA else npB
                                nc.scalar.activation(e[:, lo:hi], ps[:, lo - c:hi - c],
                                                     mybir.ActivationFunctionType.Exp,
                                                     scale=SCALE,
                                                     accum_out=parts[:, i:i + 1])
                                if qt == qtA:
                                    npA += 1
                                else:
                                    npB += 1
                        filler()
                    for qt, parts, np_ in ((qtA, partsA, npA), (qtB, partsB, npB)):
                        rsum = small.tile([128, 1], F32, tag="rsum")
                        nc.vector.reduce_sum(rsum[:], parts[:, 0:np_],
                                             axis=mybir.AxisListType.X)
                        nc.vector.reciprocal(rc[:, qt:qt + 1], rsum[:])

                # ========== E^T path ==========
                nkt = 4 * qc + 4
                ets, offs = {}, {}
                for j in range(nkt // 2):
                    ktA, ktB = 2 * j, 2 * j + 1
                    c0 = max(0, ktA * 128 - qc * CHUNK)
                    w = CHUNK - c0
                    c0B = max(0, ktB * 128 - qc * CHUNK)  # B's own start
                    wB = CHUNK - c0B
                    psA = mm_ps.tile([128, CHUNK], F32, tag="mm")
                    psB = mm_ps.tile([128, CHUNK], F32, tag="mm")
                    mA = order(nc.tensor.matmul(
                        psA[:, 0:w], kt2[0:64, ktA * 128:(ktA + 1) * 128],
                        qt2[0:64, qc * CHUNK + c0:(qc + 1) * CHUNK],
                        start=True, stop=True))
                    order(nc.tensor.matmul(
                        psB[:, 0:wB], kt2[64:128, ktB * 128:(ktB + 1) * 128],
                        qt2[64:128, qc * CHUNK + c0B:(qc + 1) * CHUNK],
                        start=True, stop=True), after=mA)
                    for kt, ps, cc, ww in ((ktA, psA, c0, w), (ktB, psB, c0B, wB)):
                        et = etpool.tile([128, CHUNK], F32R, tag="et")
                        ets[kt], offs[kt] = et, cc
                        nc.scalar.activation(et[:, cc:CHUNK], ps[:, 0:ww],
                                             mybir.ActivationFunctionType.Exp,
                                             scale=SCALE)
                        if kt * 128 >= qc * CHUNK:
                            off = kt * 128 - qc * CHUNK
                            nc.gpsimd.affine_select(
                                out=et[:, cc:CHUNK], in_=et[:, cc:CHUNK],
                                compare_op=mybir.AluOpType.is_ge, fill=0.0,
                                base=cc - off, pattern=[[1, ww]], channel_multiplier=-1)
                    filler()
                # PV chain (contiguous, warms/keeps HAM)
                po = pv_ps.tile([64, CHUNK], F32, tag="pv")
                for kt in range(nkt):
                    c0 = max(offs[kt], kt * 128 - qc * CHUNK)
                    order(nc.tensor.matmul(po[0:64, c0:CHUNK], vr[:, kt, :],
                                           ets[kt][:, c0:CHUNK],
                                           start=(kt == 0), stop=(kt == nkt - 1)))

                ot = outp.tile([64, CHUNK], F32, tag="ot")
                nc.vector.tensor_copy(ot[:], po[:])
                for jj in range(4):
                    qt = 4 * qc + jj
                    tp2 = tp_ps.tile([128, 64], F32, tag="tp")
                    order(nc.tensor.transpose(tp2[:], ot[:, jj * 128:(jj + 1) * 128],
                                              ident[0:64, 0:64]))
                    nc.vector.tensor_scalar_mul(out_sb[:, qt, :], tp2[:], rc[:, qt:qt + 1])

                for jj in range(4):
                    qt = 4 * qc + jj
                    ext = (qt + 1) * 128
                    e = e_tiles[qt]
                    nc.vector.tensor_scalar_mul(e[:, 0:ext], e[:, 0:ext],
                                                rc[:, qt:qt + 1])
                    nc.sync.dma_start(out=p_out[h, qt * 128:(qt + 1) * 128, 0:ext],
                                      in_=e[:, 0:ext])

            nc.sync.dma_start(out=o_out[h].rearrange("(n p) d -> p n d", p=128),
                              in_=out_sb[:])

    nc.compile()
    return nc


_NC_CACHE = None


def _get_nc():
    global _NC_CACHE
    if _NC_CACHE is None:
        _NC_CACHE = _build()
    return _NC_CACHE


def _run(query, key, value, trace=False):
    nc = _get_nc()
    q = np.ascontiguousarray(np.asarray(query, dtype=np.float32).reshape(B * H, S, DK))
    k = np.ascontiguousarray(np.asarray(key, dtype=np.float32).reshape(B * H, S, DK))
    v = np.ascontiguousarray(np.asarray(value, dtype=np.float32).reshape(B * H, S, DK))
    tm = np.where(np.arange(128)[None, :] <= np.arange(128)[:, None],
                  np.float32(0.0), np.float32(-1.0e9)).astype(np.float32)
    in_maps = [
        {"q": q[c * HPC:(c + 1) * HPC], "k": k[c * HPC:(c + 1) * HPC],
         "v": v[c * HPC:(c + 1) * HPC], "trimask": tm}
        for c in range(NCORES)
    ]
    res = run_bass_kernel_spmd(nc, in_maps, list(range(NCORES)), trace=trace)
    outs = np.stack([res.results[c]["o"] for c in range(NCORES)])
    ps = np.stack([res.results[c]["p_attn"] for c in range(NCORES)])
    return (outs.reshape(B, H, S, DK), ps.reshape(B, H, S, S)), res


def kernel(query, key, value, mask=None, **_ignored):
    (out, p_attn), _ = _run(query, key, value, trace=False)
    return out, p_attn
